# revision 2
# baseline (speedup 1.0000x reference)
import sys

if "/opt/trn_rl_repo" not in sys.path:
    sys.path.insert(0, "/opt/trn_rl_repo")

import numpy as np

# ---- problem constants (hardcoded per contract) ----
B, P, H, W, K = 2, 4096, 128, 128, 8
RADIUS = 0.05
R2F = float(np.float32(0.05 * 0.05))  # f32 radius^2, matches reference compare
C = 512            # candidate band capacity per pixel row
NT = B * H         # 256 row-tiles total
NCORES = 8
TPC = NT // NCORES # 32 row-tiles per core
GROUP = 4          # row-tiles per broadcast slab
BIG = 1.0e9

_PROG = {}


def _host_transform(points_world, R, T, focal):
    """Camera transform replicated bit-exactly (jax-on-cpu) vs the reference."""
    try:
        import jax
        import jax.numpy as jnp

        cpu = jax.devices("cpu")[0]
        with jax.default_device(cpu):
            pw = jnp.asarray(np.asarray(points_world, np.float32))
            Rj = jnp.asarray(np.asarray(R, np.float32))
            Tj = jnp.asarray(np.asarray(T, np.float32))
            fj = jnp.asarray(np.asarray(focal, np.float32))
            pts_view = jnp.einsum("bpi,bij->bpj", pw, Rj) + Tj[:, None, :]
            z = pts_view[..., 2]
            x_ndc = fj[:, None] * pts_view[..., 0] / z
            y_ndc = fj[:, None] * pts_view[..., 1] / z
            return (np.asarray(x_ndc), np.asarray(y_ndc), np.asarray(z))
    except Exception:
        pw = np.asarray(points_world, np.float32)
        Rm = np.asarray(R, np.float32)
        Tm = np.asarray(T, np.float32)
        f = np.asarray(focal, np.float32)
        pv = np.einsum("bpi,bij->bpj", pw, Rm) + Tm[:, None, :]
        z = pv[..., 2]
        x_ndc = f[:, None] * pv[..., 0] / z
        y_ndc = f[:, None] * pv[..., 1] / z
        return x_ndc, y_ndc, z


def _grid():
    a = np.arange(W, dtype=np.float32)
    gx = np.float32(1.0) - np.float32(2.0) * (a + np.float32(0.5)) / np.float32(W)
    b = np.arange(H, dtype=np.float32)
    gy = np.float32(1.0) - np.float32(2.0) * (b + np.float32(0.5)) / np.float32(H)
    return gx, gy


def _build_program():
    if "nc" in _PROG:
        return _PROG["nc"]
    import concourse.bacc as bacc
    import concourse.mybir as mybir
    from concourse import tile

    f32 = mybir.dt.float32
    u32 = mybir.dt.uint32
    nc = bacc.Bacc(
        "TRN2", target_bir_lowering=False, debug=False, enable_asserts=False
    )
    bands = nc.dram_tensor("bands", [1, TPC * 3 * C], f32, kind="ExternalInput")
    gyb = nc.dram_tensor("gyb", [128, TPC], f32, kind="ExternalInput")
    gxc = nc.dram_tensor("gxc", [128, 1], f32, kind="ExternalInput")
    ovals = nc.dram_tensor("ovals", [128, TPC * K], f32, kind="ExternalOutput")
    oidx = nc.dram_tensor("oidx", [128, TPC * K], u32, kind="ExternalOutput")

    GW = 3 * C * GROUP
    with tile.TileContext(nc) as tc:
        with (
            tc.tile_pool(name="const", bufs=1) as constp,
            tc.tile_pool(name="lines", bufs=2) as linep,
            tc.tile_pool(name="slabs", bufs=2) as slabp,
            tc.tile_pool(name="work", bufs=3) as workp,
            tc.tile_pool(name="acc", bufs=1) as accp,
        ):
            gyt = constp.tile([128, TPC], f32)
            nc.sync.dma_start(gyt[:, :], gyb[:, :])
            gxt = constp.tile([128, 1], f32)
            nc.sync.dma_start(gxt[:, :], gxc[:, :])
            macc = accp.tile([128, TPC * K], f32)
            iacc = accp.tile([128, TPC * K], u32)

            for grp in range(TPC // GROUP):
                line = linep.tile([1, GW], f32)
                nc.sync.dma_start(line[:, :], bands[0:1, grp * GW : (grp + 1) * GW])
                slab = slabp.tile([128, GW], f32)
                nc.gpsimd.partition_broadcast(slab[:, :], line[:, :])
                for j in range(GROUP):
                    t = grp * GROUP + j
                    base = j * 3 * C
                    xw = slab[:, base : base + C]
                    yw = slab[:, base + C : base + 2 * C]
                    nw = slab[:, base + 2 * C : base + 3 * C]
                    dx2 = workp.tile([128, C], f32, tag="dx2")
                    nc.scalar.activation(
                        dx2[:, :],
                        xw,
                        mybir.ActivationFunctionType.Square,
                        bias=gxt[:, 0:1],
                        scale=-1.0,
                    )
                    dy2 = workp.tile([128, C], f32, tag="dy2")
                    nc.scalar.activation(
                        dy2[:, :],
                        yw,
                        mybir.ActivationFunctionType.Square,
                        bias=gyt[:, t : t + 1],
                        scale=-1.0,
                    )
                    s = workp.tile([128, C], f32, tag="s")
                    nc.vector.tensor_add(s[:, :], dx2[:, :], dy2[:, :])
                    m = workp.tile([128, C], f32, tag="m")
                    nc.vector.tensor_scalar(
                        m[:, :],
                        s[:, :],
                        R2F,
                        -BIG,
                        mybir.AluOpType.is_ge,
                        mybir.AluOpType.mult,
                    )
                    key = workp.tile([128, C], f32, tag="key")
                    nc.vector.tensor_add(key[:, :], m[:, :], nw)
                    mx = workp.tile([128, K], f32, tag="mx")
                    nc.vector.max(mx[:, :], key[:, :])
                    nc.vector.max_index(
                        iacc[:, t * K : (t + 1) * K], mx[:, :], key[:, :]
                    )
                    nc.vector.tensor_copy(macc[:, t * K : (t + 1) * K], mx[:, :])

            zacc = accp.tile([128, TPC * K], f32)
            nc.scalar.mul(zacc[:, :], macc[:, :], -1.0)
            nc.sync.dma_start(ovals[:, :], zacc[:, :])
            nc.sync.dma_start(oidx[:, :], iacc[:, :])

    nc.compile()
    _PROG["nc"] = nc
    return nc


def _run(points_world, R, T, focal, trace=False):
    from concourse.bass_utils import run_bass_kernel_spmd

    points_world = np.asarray(points_world, np.float32)
    R = np.asarray(R, np.float32)
    T = np.asarray(T, np.float32)
    focal = np.asarray(focal, np.float32)

    x_ndc, y_ndc, z = _host_transform(points_world, R, T, focal)
    gx, gy = _grid()

    # per-batch: drop z<=0 points, sort by y_ndc
    sids, xs_s, ys_s, zs_s = [], [], [], []
    for b in range(B):
        ids = np.nonzero(z[b] > 0.0)[0]
        order = np.argsort(y_ndc[b][ids], kind="stable")
        sid = ids[order]
        sids.append(sid)
        xs_s.append(x_ndc[b][sid])
        ys_s.append(y_ndc[b][sid])
        zs_s.append(z[b][sid])
        assert len(sid) >= C

    rpad = float(np.float64(np.float32(RADIUS)) * (1.0 + 1e-5))
    los = np.zeros(NT, np.int64)
    in_maps = []
    for c in range(NCORES):
        bands = np.empty((TPC, 3, C), np.float32)
        gyb = np.empty((128, TPC), np.float32)
        for t in range(TPC):
            g = c * TPC + t
            b, h = g // H, g % H
            ys = ys_s[b]
            gy_h = np.float64(gy[h])
            lo = np.searchsorted(ys, gy_h - rpad, side="left")
            hi = np.searchsorted(ys, gy_h + rpad, side="right")
            assert hi - lo <= C, f"band overflow: {hi - lo}"
            lo_c = min(int(lo), len(ys) - C)
            los[g] = lo_c
            bands[t, 0] = xs_s[b][lo_c : lo_c + C]
            bands[t, 1] = ys[lo_c : lo_c + C]
            bands[t, 2] = -zs_s[b][lo_c : lo_c + C]
            gyb[:, t] = gy[h]
        in_maps.append(
            {
                "bands": bands.reshape(1, TPC * 3 * C),
                "gyb": gyb,
                "gxc": gx.reshape(128, 1).copy(),
            }
        )

    global _last_in_maps
    _last_in_maps = in_maps
    nc = _build_program()
    res = run_bass_kernel_spmd(
        nc, in_maps, core_ids=list(range(NCORES)), trace=trace
    )

    idx = np.full((B, H, W, K), -1, np.int32)
    zbuf = np.full((B, H, W, K), -1.0, np.float32)
    dists = np.full((B, H, W, K), -1.0, np.float32)
    for g in range(NT):
        c, t = g // TPC, g % TPC
        b, h = g // H, g % H
        zr = np.asarray(res.results[c]["ovals"])[:, t * K : (t + 1) * K]
        ir = np.asarray(res.results[c]["oidx"])[:, t * K : (t + 1) * K]
        ir = ir.astype(np.int64)
        val = zr < BIG / 2
        ir = np.where(val & (ir >= 0) & (ir < C), ir, 0)
        gid = sids[b][los[g] + ir]  # [W, K] original point ids
        xk = x_ndc[b][gid]
        yk = y_ndc[b][gid]
        dxk = gx[:, None] - xk
        dyk = gy[h] - yk
        dk = dxk * dxk + dyk * dyk
        idx[b, h] = np.where(val, gid.astype(np.int32), -1)
        zbuf[b, h] = np.where(val, zr, np.float32(-1.0))
        dists[b, h] = np.where(val, dk.astype(np.float32), np.float32(-1.0))

    return (idx, zbuf, dists), res


def kernel(points_world, R, T, focal):
    out, _ = _run(points_world, R, T, focal, trace=False)
    return out


# revision 5
# speedup vs baseline: 456.6889x; 456.6889x over previous
import sys

if "/opt/trn_rl_repo" not in sys.path:
    sys.path.insert(0, "/opt/trn_rl_repo")

import numpy as np

# ---- problem constants (hardcoded per contract) ----
B, P, H, W, K = 2, 4096, 128, 128, 8
RADIUS = 0.05
R2F = float(np.float32(0.05 * 0.05))  # f32 radius^2, matches reference compare
C = 512            # candidate band capacity per pixel row
NT = B * H         # 256 row-tiles total
NCORES = 8
TPC = NT // NCORES # 32 row-tiles per core
GROUP = 4          # row-tiles per broadcast slab
BIG = 1.0e9

_PROG = {}


def _host_transform(points_world, R, T, focal):
    """Camera transform replicated bit-exactly (jax-on-cpu) vs the reference."""
    try:
        import jax
        import jax.numpy as jnp

        cpu = jax.devices("cpu")[0]
        with jax.default_device(cpu):
            pw = jnp.asarray(np.asarray(points_world, np.float32))
            Rj = jnp.asarray(np.asarray(R, np.float32))
            Tj = jnp.asarray(np.asarray(T, np.float32))
            fj = jnp.asarray(np.asarray(focal, np.float32))
            pts_view = jnp.einsum("bpi,bij->bpj", pw, Rj) + Tj[:, None, :]
            z = pts_view[..., 2]
            x_ndc = fj[:, None] * pts_view[..., 0] / z
            y_ndc = fj[:, None] * pts_view[..., 1] / z
            return (np.asarray(x_ndc), np.asarray(y_ndc), np.asarray(z))
    except Exception:
        pw = np.asarray(points_world, np.float32)
        Rm = np.asarray(R, np.float32)
        Tm = np.asarray(T, np.float32)
        f = np.asarray(focal, np.float32)
        pv = np.einsum("bpi,bij->bpj", pw, Rm) + Tm[:, None, :]
        z = pv[..., 2]
        x_ndc = f[:, None] * pv[..., 0] / z
        y_ndc = f[:, None] * pv[..., 1] / z
        return x_ndc, y_ndc, z


def _grid():
    a = np.arange(W, dtype=np.float32)
    gx = np.float32(1.0) - np.float32(2.0) * (a + np.float32(0.5)) / np.float32(W)
    b = np.arange(H, dtype=np.float32)
    gy = np.float32(1.0) - np.float32(2.0) * (b + np.float32(0.5)) / np.float32(H)
    return gx, gy


def _build_program(reps=1):
    if reps in _PROG:
        return _PROG[reps]
    import concourse.bacc as bacc
    import concourse.mybir as mybir
    from concourse import tile

    f32 = mybir.dt.float32
    u32 = mybir.dt.uint32
    nc = bacc.Bacc(
        "TRN2", target_bir_lowering=False, debug=False, enable_asserts=False
    )
    bands = nc.dram_tensor("bands", [1, TPC * 3 * C], f32, kind="ExternalInput")
    gyb = nc.dram_tensor("gyb", [128, TPC], f32, kind="ExternalInput")
    gxc = nc.dram_tensor("gxc", [128, 1], f32, kind="ExternalInput")
    ovals = nc.dram_tensor("ovals", [128, TPC * K], f32, kind="ExternalOutput")
    oidx = nc.dram_tensor("oidx", [128, TPC * K], u32, kind="ExternalOutput")

    GW = 3 * C * GROUP
    with tile.TileContext(nc) as tc:
        with (
            tc.tile_pool(name="const", bufs=1) as constp,
            tc.tile_pool(name="lines", bufs=2) as linep,
            tc.tile_pool(name="slabs", bufs=2) as slabp,
            tc.tile_pool(name="work", bufs=3) as workp,
            tc.tile_pool(name="acc", bufs=1) as accp,
        ):
            gyt = constp.tile([128, TPC], f32)
            nc.sync.dma_start(gyt[:, :], gyb[:, :])
            gxt = constp.tile([128, 1], f32)
            nc.sync.dma_start(gxt[:, :], gxc[:, :])
            macc = accp.tile([128, TPC * K], f32)
            iacc = accp.tile([128, TPC * K], u32)

            for rep in range(reps):
              for grp in range(TPC // GROUP):
                line = linep.tile([1, GW], f32)
                nc.sync.dma_start(line[:, :], bands[0:1, grp * GW : (grp + 1) * GW])
                slab = slabp.tile([128, GW], f32)
                nc.gpsimd.partition_broadcast(slab[:, :], line[:, :])
                for j in range(GROUP):
                    t = grp * GROUP + j
                    base = j * 3 * C
                    xw = slab[:, base : base + C]
                    yw = slab[:, base + C : base + 2 * C]
                    nw = slab[:, base + 2 * C : base + 3 * C]
                    dx2 = workp.tile([128, C], f32, tag="dx2")
                    nc.scalar.activation(
                        dx2[:, :],
                        xw,
                        mybir.ActivationFunctionType.Square,
                        bias=gxt[:, 0:1],
                        scale=-1.0,
                    )
                    dy2 = workp.tile([128, C], f32, tag="dy2")
                    nc.scalar.activation(
                        dy2[:, :],
                        yw,
                        mybir.ActivationFunctionType.Square,
                        bias=gyt[:, t : t + 1],
                        scale=-1.0,
                    )
                    s = workp.tile([128, C], f32, tag="s")
                    nc.vector.tensor_add(s[:, :], dx2[:, :], dy2[:, :])
                    m = workp.tile([128, C], f32, tag="m")
                    nc.vector.tensor_scalar(
                        m[:, :],
                        s[:, :],
                        R2F,
                        -BIG,
                        mybir.AluOpType.is_ge,
                        mybir.AluOpType.mult,
                    )
                    key = workp.tile([128, C], f32, tag="key")
                    nc.vector.tensor_add(key[:, :], m[:, :], nw)
                    mx = workp.tile([128, K], f32, tag="mx")
                    nc.vector.max(mx[:, :], key[:, :])
                    nc.vector.max_index(
                        iacc[:, t * K : (t + 1) * K], mx[:, :], key[:, :]
                    )
                    nc.vector.tensor_copy(macc[:, t * K : (t + 1) * K], mx[:, :])

            zacc = accp.tile([128, TPC * K], f32)
            nc.scalar.mul(zacc[:, :], macc[:, :], -1.0)
            nc.sync.dma_start(ovals[:, :], zacc[:, :])
            nc.sync.dma_start(oidx[:, :], iacc[:, :])

    nc.compile()
    _PROG[reps] = nc
    return nc


def _run(points_world, R, T, focal, trace=False):
    from concourse.bass_utils import run_bass_kernel_spmd

    points_world = np.asarray(points_world, np.float32)
    R = np.asarray(R, np.float32)
    T = np.asarray(T, np.float32)
    focal = np.asarray(focal, np.float32)

    x_ndc, y_ndc, z = _host_transform(points_world, R, T, focal)
    gx, gy = _grid()

    # per-batch: drop z<=0 points, sort by y_ndc
    sids, xs_s, ys_s, zs_s = [], [], [], []
    for b in range(B):
        ids = np.nonzero(z[b] > 0.0)[0]
        order = np.argsort(y_ndc[b][ids], kind="stable")
        sid = ids[order]
        sids.append(sid)
        xs_s.append(x_ndc[b][sid])
        ys_s.append(y_ndc[b][sid])
        zs_s.append(z[b][sid])
        assert len(sid) >= C

    rpad = float(np.float64(np.float32(RADIUS)) * (1.0 + 1e-5))
    los = np.zeros(NT, np.int64)
    in_maps = []
    for c in range(NCORES):
        bands = np.empty((TPC, 3, C), np.float32)
        gyb = np.empty((128, TPC), np.float32)
        for t in range(TPC):
            g = c * TPC + t
            b, h = g // H, g % H
            ys = ys_s[b]
            gy_h = np.float64(gy[h])
            lo = np.searchsorted(ys, gy_h - rpad, side="left")
            hi = np.searchsorted(ys, gy_h + rpad, side="right")
            assert hi - lo <= C, f"band overflow: {hi - lo}"
            lo_c = min(int(lo), len(ys) - C)
            los[g] = lo_c
            bands[t, 0] = xs_s[b][lo_c : lo_c + C]
            bands[t, 1] = ys[lo_c : lo_c + C]
            bands[t, 2] = -zs_s[b][lo_c : lo_c + C]
            gyb[:, t] = gy[h]
        in_maps.append(
            {
                "bands": bands.reshape(1, TPC * 3 * C),
                "gyb": gyb,
                "gxc": gx.reshape(128, 1).copy(),
            }
        )

    global _last_in_maps
    _last_in_maps = in_maps
    nc = _build_program()
    res = run_bass_kernel_spmd(
        nc, in_maps, core_ids=list(range(NCORES)), trace=trace
    )

    idx = np.full((B, H, W, K), -1, np.int32)
    zbuf = np.full((B, H, W, K), -1.0, np.float32)
    dists = np.full((B, H, W, K), -1.0, np.float32)
    for g in range(NT):
        c, t = g // TPC, g % TPC
        b, h = g // H, g % H
        zr = np.asarray(res.results[c]["ovals"])[:, t * K : (t + 1) * K]
        ir = np.asarray(res.results[c]["oidx"])[:, t * K : (t + 1) * K]
        ir = ir.astype(np.int64)
        val = zr < BIG / 2
        ir = np.where(val & (ir >= 0) & (ir < C), ir, 0)
        gid = sids[b][los[g] + ir]  # [W, K] original point ids
        xk = x_ndc[b][gid]
        yk = y_ndc[b][gid]
        dxk = gx[:, None] - xk
        dyk = gy[h] - yk
        dk = dxk * dxk + dyk * dyk
        idx[b, h] = np.where(val, gid.astype(np.int32), -1)
        zbuf[b, h] = np.where(val, zr, np.float32(-1.0))
        dists[b, h] = np.where(val, dk.astype(np.float32), np.float32(-1.0))

    return (idx, zbuf, dists), res


def kernel(points_world, R, T, focal):
    out, _ = _run(points_world, R, T, focal, trace=False)
    return out


# revision 16
# speedup vs baseline: 964.0518x; 2.1110x over previous
import sys

if "/opt/trn_rl_repo" not in sys.path:
    sys.path.insert(0, "/opt/trn_rl_repo")

import numpy as np

# ---- problem constants (hardcoded per contract) ----
B, P, H, W, K = 2, 4096, 128, 128, 8
RADIUS = 0.05
R2F = float(np.float32(0.05 * 0.05))  # f32 radius^2, matches reference compare
R2M = float(np.nextafter(np.float32(R2F), np.float32(0.0)))  # largest f32 < R2F
NT = B * H         # 256 row-tiles total
NCORES = 8
TPC = NT // NCORES # 32 row-tiles per core
GROUP = 2          # rows sharing one broadcast slab
SCALE = 1.0e16     # outside-penalty scale: SCALE*ulp(R2F) ~ 2.3e6
VALTH = 1.0e6      # recovered z below this => inside

_PROG = {}


def _host_transform(points_world, R, T, focal):
    """Camera transform replicated bit-exactly (jax-on-cpu) vs the reference."""
    try:
        import jax
        import jax.numpy as jnp

        cpu = jax.devices("cpu")[0]
        with jax.default_device(cpu):
            pw = jnp.asarray(np.asarray(points_world, np.float32))
            Rj = jnp.asarray(np.asarray(R, np.float32))
            Tj = jnp.asarray(np.asarray(T, np.float32))
            fj = jnp.asarray(np.asarray(focal, np.float32))
            pts_view = jnp.einsum("bpi,bij->bpj", pw, Rj) + Tj[:, None, :]
            z = pts_view[..., 2]
            x_ndc = fj[:, None] * pts_view[..., 0] / z
            y_ndc = fj[:, None] * pts_view[..., 1] / z
            return (np.asarray(x_ndc), np.asarray(y_ndc), np.asarray(z))
    except Exception:
        pw = np.asarray(points_world, np.float32)
        Rm = np.asarray(R, np.float32)
        Tm = np.asarray(T, np.float32)
        f = np.asarray(focal, np.float32)
        pv = np.einsum("bpi,bij->bpj", pw, Rm) + Tm[:, None, :]
        z = pv[..., 2]
        x_ndc = f[:, None] * pv[..., 0] / z
        y_ndc = f[:, None] * pv[..., 1] / z
        return x_ndc, y_ndc, z


def _grid():
    a = np.arange(W, dtype=np.float32)
    gx = np.float32(1.0) - np.float32(2.0) * (a + np.float32(0.5)) / np.float32(W)
    b = np.arange(H, dtype=np.float32)
    gy = np.float32(1.0) - np.float32(2.0) * (b + np.float32(0.5)) / np.float32(H)
    return gx, gy


def _build_program(reps=1, no_bcast=False, no_compute=False, C=512, group=GROUP):
    pkey = (reps, no_bcast, no_compute, C, group)
    if pkey in _PROG:
        return _PROG[pkey]
    import concourse.bacc as bacc
    import concourse.mybir as mybir
    from concourse import tile

    f32 = mybir.dt.float32
    u32 = mybir.dt.uint32
    NG = TPC // group  # broadcast groups per core
    GW = 3 * C         # slab width per group: x | y | negz
    nc = bacc.Bacc(
        "TRN2", target_bir_lowering=False, debug=False, enable_asserts=False
    )
    bands = nc.dram_tensor("bands", [1, NG * GW], f32, kind="ExternalInput")
    gyb = nc.dram_tensor("gyb", [128, TPC], f32, kind="ExternalInput")
    gxc = nc.dram_tensor("gxc", [128, 1], f32, kind="ExternalInput")
    ovals = nc.dram_tensor("ovals", [128, TPC * K], f32, kind="ExternalOutput")
    oidx = nc.dram_tensor("oidx", [128, TPC * K], u32, kind="ExternalOutput")

    with tile.TileContext(nc) as tc:
        with (
            tc.tile_pool(name="const", bufs=1) as constp,
            tc.tile_pool(name="lines", bufs=3) as linep,
            tc.tile_pool(name="slabs", bufs=3) as slabp,
            tc.tile_pool(name="work", bufs=3) as workp,
            tc.tile_pool(name="acc", bufs=1) as accp,
        ):
            gyt = constp.tile([128, TPC], f32)
            nc.sync.dma_start(gyt[:, :], gyb[:, :])
            gxt = constp.tile([128, 1], f32)
            nc.sync.dma_start(gxt[:, :], gxc[:, :])
            r2t = constp.tile([128, 1], f32)
            nc.gpsimd.memset(r2t[:, :], -R2M)
            macc = accp.tile([128, TPC * K], f32)
            iacc = accp.tile([128, TPC * K], u32)

            for rep in range(reps):
              for grp in range(NG):
                line = linep.tile([1, GW], f32)
                nc.sync.dma_start(line[:, :], bands[0:1, grp * GW : (grp + 1) * GW])
                slab = slabp.tile([128, GW], f32)
                if no_bcast:
                    nc.gpsimd.partition_broadcast(slab[:, 0:8], line[:, 0:8])
                else:
                    nc.gpsimd.partition_broadcast(slab[:, :], line[:, :])
                if no_compute:
                    continue
                xw = slab[:, 0:C]
                nw = slab[:, 2 * C : 3 * C]
                dx2 = workp.tile([128, C], f32, tag="dx2")
                nc.scalar.activation(
                    dx2[:, :],
                    xw,
                    mybir.ActivationFunctionType.Square,
                    bias=gxt[:, 0:1],
                    scale=-1.0,
                )
                sbig = workp.tile([128, group * C], f32, tag="sbig")
                dy2 = workp.tile([128, group * C], f32, tag="dy2")
                for j in range(group):
                    t = grp * group + j
                    nc.scalar.activation(
                        dy2[:, j * C : (j + 1) * C],
                        slab[:, C : 2 * C],
                        mybir.ActivationFunctionType.Square,
                        bias=gyt[:, t : t + 1],
                        scale=-1.0,
                    )
                    nc.vector.tensor_add(
                        sbig[:, j * C : (j + 1) * C],
                        dx2[:, :],
                        dy2[:, j * C : (j + 1) * C],
                    )
                rbig = workp.tile([128, group * C], f32, tag="rbig")
                nc.scalar.activation(
                    rbig[:, :],
                    sbig[:, :],
                    mybir.ActivationFunctionType.Relu,
                    bias=r2t[:, 0:1],
                    scale=1.0,
                )
                kbig = workp.tile([128, group * C], f32, tag="kbig")
                for j in range(group):
                    t = grp * group + j
                    nc.vector.scalar_tensor_tensor(
                        kbig[:, j * C : (j + 1) * C],
                        rbig[:, j * C : (j + 1) * C],
                        -SCALE,
                        nw,
                        mybir.AluOpType.mult,
                        mybir.AluOpType.add,
                    )
                    nc.vector.max(
                        macc[:, t * K : (t + 1) * K], kbig[:, j * C : (j + 1) * C]
                    )
                    nc.vector.max_index(
                        iacc[:, t * K : (t + 1) * K],
                        macc[:, t * K : (t + 1) * K],
                        kbig[:, j * C : (j + 1) * C],
                    )

            if not no_compute:
                zacc = accp.tile([128, TPC * K], f32)
                nc.scalar.mul(zacc[:, :], macc[:, :], -1.0)
                nc.sync.dma_start(ovals[:, :], zacc[:, :])
                nc.sync.dma_start(oidx[:, :], iacc[:, :])

    nc.compile()
    _PROG[pkey] = nc
    return nc


def _run(points_world, R, T, focal, trace=False):
    from concourse.bass_utils import run_bass_kernel_spmd

    points_world = np.asarray(points_world, np.float32)
    R = np.asarray(R, np.float32)
    T = np.asarray(T, np.float32)
    focal = np.asarray(focal, np.float32)

    x_ndc, y_ndc, z = _host_transform(points_world, R, T, focal)
    gx, gy = _grid()

    # per-batch: drop z<=0 points, sort by y_ndc
    sids, xs_s, ys_s, zs_s = [], [], [], []
    for b in range(B):
        ids = np.nonzero(z[b] > 0.0)[0]
        order = np.argsort(y_ndc[b][ids], kind="stable")
        sid = ids[order]
        sids.append(sid)
        xs_s.append(x_ndc[b][sid])
        ys_s.append(y_ndc[b][sid])
        zs_s.append(z[b][sid])

    # band capacity: widest per-group union band, rounded up to 128 (min 256)
    rpad = float(np.float64(np.float32(RADIUS)) * (1.0 + 1e-5))
    NGT = NT // GROUP  # total groups across cores
    need = 1
    for gg in range(NGT):
        g0 = gg * GROUP
        b = g0 // H
        h0, h1 = g0 % H, (g0 + GROUP - 1) % H
        gy_max = np.float64(gy[h0])   # gy decreasing in h
        gy_min = np.float64(gy[h1])
        lo = np.searchsorted(ys_s[b], gy_min - rpad, side="left")
        hi = np.searchsorted(ys_s[b], gy_max + rpad, side="right")
        need = max(need, int(hi - lo))
    Cn = max(256, ((need + 127) // 128) * 128)
    global _last_C
    _last_C = Cn

    # pad to >= Cn with far-away sentinel points (never inside radius)
    for b in range(B):
        npad = Cn - len(sids[b])
        if npad > 0:
            sids[b] = np.concatenate([sids[b], np.zeros(npad, sids[b].dtype)])
            xs_s[b] = np.concatenate([xs_s[b], np.full(npad, 1e3, np.float32)])
            ys_s[b] = np.concatenate([ys_s[b], np.full(npad, 1e3, np.float32)])
            zs_s[b] = np.concatenate([zs_s[b], np.ones(npad, np.float32)])

    NG = TPC // GROUP  # groups per core
    los = np.zeros(NT, np.int64)
    in_maps = []
    for c in range(NCORES):
        bands = np.empty((NG, 3, Cn), np.float32)
        gyb = np.empty((128, TPC), np.float32)
        for grp in range(NG):
            g0 = c * TPC + grp * GROUP
            b = g0 // H
            h0, h1 = g0 % H, (g0 + GROUP - 1) % H
            ys = ys_s[b]
            lo = np.searchsorted(ys, np.float64(gy[h1]) - rpad, side="left")
            lo_c = min(int(lo), len(ys) - Cn)
            bands[grp, 0] = xs_s[b][lo_c : lo_c + Cn]
            bands[grp, 1] = ys[lo_c : lo_c + Cn]
            bands[grp, 2] = -zs_s[b][lo_c : lo_c + Cn]
            for j in range(GROUP):
                t = grp * GROUP + j
                los[c * TPC + t] = lo_c
                gyb[:, t] = gy[(g0 + j) % H]
        in_maps.append(
            {
                "bands": bands.reshape(1, NG * 3 * Cn),
                "gyb": gyb,
                "gxc": gx.reshape(128, 1).copy(),
            }
        )

    global _last_in_maps
    _last_in_maps = in_maps
    nc = _build_program(C=Cn)
    res = run_bass_kernel_spmd(
        nc, in_maps, core_ids=list(range(NCORES)), trace=trace
    )

    idx = np.full((B, H, W, K), -1, np.int32)
    zbuf = np.full((B, H, W, K), -1.0, np.float32)
    dists = np.full((B, H, W, K), -1.0, np.float32)
    for g in range(NT):
        c, t = g // TPC, g % TPC
        b, h = g // H, g % H
        zr = np.asarray(res.results[c]["ovals"])[:, t * K : (t + 1) * K]
        ir = np.asarray(res.results[c]["oidx"])[:, t * K : (t + 1) * K]
        ir = ir.astype(np.int64)
        val = zr < VALTH
        ir = np.where(val & (ir >= 0) & (ir < Cn), ir, 0)
        gid = sids[b][los[g] + ir]  # [W, K] original point ids
        xk = x_ndc[b][gid]
        yk = y_ndc[b][gid]
        dxk = gx[:, None] - xk
        dyk = gy[h] - yk
        dk = dxk * dxk + dyk * dyk
        idx[b, h] = np.where(val, gid.astype(np.int32), -1)
        zbuf[b, h] = np.where(val, zr, np.float32(-1.0))
        dists[b, h] = np.where(val, dk.astype(np.float32), np.float32(-1.0))

    return (idx, zbuf, dists), res


def kernel(points_world, R, T, focal):
    out, _ = _run(points_world, R, T, focal, trace=False)
    return out
